# Initial kernel scaffold
#
"""Multi-head attention forward on 8 Trainium2 NeuronCores (Bass/Tile).

Problem: B=4, T=2048, D=512, H=8, HS=64, fp32.
  q/k/v = einsum('btd,hde->bhte', x, W{q,k,v})
  att   = softmax(q k^T / sqrt(HS))
  out   = (att v) concat-heads @ Wo + bo

Sharding (8 cores): core c -> batch b=c//2, heads hb=4*(c%2)..hb+4
(data parallel on B x tensor parallel on H). Each core computes its 4 heads'
attention and a partial output projection against its 256 rows of Wo (bias
halved per core); the host sums the two partials per batch.

On-device dataflow per core (all matmuls in float16: 1 cycle/row on PE,
~1e-3 rel err; PSUM accumulation is fp32):
  phase 1: qT/kT per head-pair [128=2*HS, T] and v [T, 4*(HS+1)] (ones column
           appended per head for the softmax denominator) from xT [D, T].
  phase 2: per head-pair, per 512-wide t-chunk, loop s-tiles of 128:
           ST[s,t] matmuls (K=HS=64, two heads row-packed at partitions 0/64),
           one ScalarE exp over [128, 1024] PSUM (scale=1/8 folded in),
           AV matmuls accumulate [65, 512] (row 64 = sum of exp).
           Then normalize: reciprocal of row 64, partition-broadcast,
           multiply -> outT [4*HS, T] fp32r.
  phase 3: y[t,:] = outT.T @ Wo_rows + 0.5*bo via 3 accumulating matmuls
           (ones-row trick for the bias), DVE copy, DMA out.
"""
import os
import sys

sys.path.insert(0, "/opt/trn_rl_repo")

import numpy as np
from contextlib import ExitStack

import concourse.bacc as bacc
import concourse.tile as tile
from concourse import mybir
from concourse.bass_utils import run_bass_kernel_spmd

B, T, D, H, HS = 4, 2048, 512, 8, 64
NCORES = 8
P = 128
HPC = 4  # heads per core
F32 = mybir.dt.float32
F32R = mybir.dt.float32r
F16 = mybir.dt.float16
EXP = mybir.ActivationFunctionType.Exp


def to_fp16(x: np.ndarray) -> np.ndarray:
    return np.ascontiguousarray(np.asarray(x, dtype=np.float32).astype(np.float16))


def _emit(tc, xT, wq, wk, wv, wo, bo2, y):
    nc = tc.nc
    with ExitStack() as ctx:
        persist = ctx.enter_context(tc.tile_pool(name="persist", bufs=1))

        # ---- persistent SBUF tiles ----
        xt_sb = [persist.tile([P, T], F16, tag=f"xt{i}", name=f"xt{i}") for i in range(4)]
        wq_sb = [persist.tile([P, 2 * P], F16, tag=f"wq{i}", name=f"wq{i}") for i in range(4)]
        wk_sb = [persist.tile([P, 2 * P], F16, tag=f"wk{i}", name=f"wk{i}") for i in range(4)]
        wv_sb = [persist.tile([P, 2 * P], F16, tag=f"wv{i}", name=f"wv{i}") for i in range(4)]
        wo_sb = [persist.tile([P, D], F16, tag=f"wo{i}", name=f"wo{i}") for i in range(2)]
        bo2_sb = persist.tile([1, D], F32, tag="bo2")
        bo_bc = persist.tile([P, D], F32, tag="bo_bc")
        q2 = [persist.tile([P, T], F16, tag=f"q2{i}", name=f"q2_{i}") for i in range(2)]
        k2 = [persist.tile([P, T], F16, tag=f"k2{i}", name=f"k2_{i}") for i in range(2)]
        v_sb = [persist.tile([P, HPC * (HS + 1)], F16, tag=f"v{i}", name=f"v{i}") for i in range(16)]
        out2 = [persist.tile([P, T], F16, tag=f"o2{i}", name=f"o2_{i}") for i in range(2)]
        ones_v16 = persist.tile([P, HPC], F16, tag="ones_v16")
        warm_in = persist.tile([P, 512], F16, tag="warm_in")

        # PE warm-up: ~3.5us of dummy matmuls during the DMA lead-in flips
        # the HAM clock gate to 2.4 GHz before the real matmuls start.
        with tc.tile_pool(name="ps_warm", bufs=1, space="PSUM") as ps_warm:
            nc.vector.memset(warm_in, 0.5)
            wp = ps_warm.tile([P, 512], F32, tag="warm")
            for _ in range(8):
                nc.tensor.matmul(wp, warm_in[:, 0:P], warm_in, start=True, stop=True)

        # input DMAs, spread across engine queues so issue doesn't serialize.
        # xT is streamed per 512-column chunk so the first projection group
        # (which contracts over all four d-tiles) can start after only the
        # four chunk-0 pieces have landed.
        for tch in range(4):
            csl = slice(tch * 512, (tch + 1) * 512)
            for i in range(4):
                dsl = slice(i * P, (i + 1) * P)
                nc.sync.dma_start(out=xt_sb[i][:, csl], in_=xT[dsl, csl])
        for i in range(4):
            dsl = slice(i * P, (i + 1) * P)
            nc.scalar.dma_start(out=wq_sb[i], in_=wq[dsl, :])
            nc.scalar.dma_start(out=wk_sb[i], in_=wk[dsl, :])
            nc.gpsimd.dma_start(out=wv_sb[i], in_=wv[dsl, :])
        for i in range(2):
            nc.gpsimd.dma_start(out=wo_sb[i], in_=wo[i * P : (i + 1) * P, :])
        nc.gpsimd.dma_start(out=bo2_sb, in_=bo2)
        nc.gpsimd.partition_broadcast(bo_bc, bo2_sb)
        nc.vector.memset(ones_v16, 1.0)
        # staging tile for the softmax-denominator rows (partitions 0 and
        # 32); filler rows preset to 1.0 so Ln/Exp of the unused lanes stay
        # finite
        srow_p = persist.tile([33, 512], F32, tag="srow_p")
        nc.vector.memset(srow_p, 1.0)

        # One shared PSUM layout for everything: st 2x[128,1024] (4 banks) +
        # 3 general [128,512] slots (tag "av": projection groups AND the AV
        # accumulators) + 1 y slot = 8 banks.
        with (
            tc.tile_pool(name="ps_st", bufs=2, space="PSUM") as ps_st,
            tc.tile_pool(name="ps_av", bufs=4, space="PSUM") as ps_av,
            tc.tile_pool(name="attp", bufs=6) as attp,
            tc.tile_pool(name="nrm", bufs=6) as nrm,
            tc.tile_pool(name="yout", bufs=3) as yout,
        ):
            def emit_qk_group(w_sb, dst, pr, tch, eng=None):
                # one [128,512] chunk of the q or k projection (4 accumulating
                # matmuls over D, then a cast-copy to fp16 SBUF)
                psl = slice(pr * P, (pr + 1) * P)
                tsl = slice(tch * 512, (tch + 1) * 512)
                pt = ps_av.tile(
                    [P, 512], F32, tag="av", name=f"qk{id(dst)}_{pr}_{tch}"
                )
                for di in range(4):
                    nc.tensor.matmul(
                        pt,
                        w_sb[di][:, psl],
                        xt_sb[di][:, tsl],
                        start=(di == 0),
                        stop=(di == 3),
                    )
                nc.vector.tensor_copy(dst[pr][:, tsl], pt)

            def emit_v_group(tt):
                ttsl = slice(tt * P, (tt + 1) * P)
                pv = ps_av.tile([P, 2 * P], F32, tag="av", name=f"pv{tt}")
                for di in range(4):
                    nc.tensor.matmul(
                        pv,
                        xt_sb[di][:, ttsl],
                        wv_sb[di],
                        start=(di == 0),
                        stop=(di == 3),
                    )
                v3 = v_sb[tt].rearrange("p (h e) -> p h e", h=HPC)
                nc.vector.tensor_copy(
                    v3[:, :, 0:HS], pv.rearrange("p (h e) -> p h e", h=HPC)
                )
                nc.vector.tensor_copy(v3[:, :, HS], ones_v16)

            def emit_proj(tt):
                # output projection t-tile + bias add (from broadcast bo_bc)
                ttsl = slice(tt * P, (tt + 1) * P)
                yp = ps_av.tile([P, D], F32, tag="av", name=f"yp{tt}")
                nc.tensor.matmul(yp, out2[0][:, ttsl], wo_sb[0], start=True, stop=False)
                nc.tensor.matmul(yp, out2[1][:, ttsl], wo_sb[1], start=False, stop=True)
                ys = yout.tile([P, D], F32, tag="y")
                nc.vector.tensor_add(ys, yp, bo_bc)
                nc.sync.dma_start(out=y[ttsl, :], in_=ys)

            def emit_norm(av, hp, tq):
                # divide the pair's unnormalized outputs by their sums of
                # exp: stage both [1,512] sum rows side by side (DVE), one
                # Ln + one Exp(-x) over [1,1024] (ScalarE ops batched - it is
                # the critical engine), partition-broadcast on GpSimd,
                # multiply on DVE. Runs as deferred filler, so inputs are
                # ready and nothing stalls.
                tsl = slice(tq * 512, (tq + 1) * 512)
                # batch the pair's Ln/Exp on partitions 0 and 32 (ScalarE
                # cost scales with free-size per partition: one [33,512] op
                # costs what a [1,512] op does). Tile misses the dependency
                # from the single-row staging copies to the full-tile Ln
                # read, so add it explicitly.
                cps = [
                    nc.vector.tensor_copy(
                        srow_p[32 * j : 32 * j + 1, :], av[j][HS : HS + 1, :]
                    )
                    for j in range(2)
                ]
                lnr = nrm.tile([33, 512], F32, tag="lnr", name=f"lnr{hp}_{tq}")
                li = nc.scalar.activation(
                    lnr, srow_p, func=mybir.ActivationFunctionType.Ln
                )
                for c in cps:
                    tile.add_dep_helper(li.ins, c.ins, reason="srow staging")
                recip = nrm.tile([33, 512], F32, tag="recip")
                nc.scalar.activation(recip, lnr, func=EXP, scale=-1.0)
                # broadcast only from partition-0 sources (quadrant reach):
                # copy row 32 down first
                recip1 = nrm.tile([1, 512], F32, tag="recip1", name=f"rc1{hp}_{tq}")
                nc.vector.tensor_copy(recip1, recip[32:33, :])
                for j, rsrc in ((0, recip[0:1, :]), (1, recip1[:, :])):
                    bco = nrm.tile([HS, 512], F32, tag="bco", name=f"bco{hp}_{tq}_{j}")
                    nc.gpsimd.partition_broadcast(bco, rsrc)
                    nc.vector.tensor_mul(
                        out2[hp][j * HS : (j + 1) * HS, tsl],
                        av[j][0:HS, :],
                        bco,
                    )

            def emit_st(hp, tq, si):
                # scores for both heads of the pair, row-packed at
                # partitions 0 / 64 (K=64 each) -> concurrent on the array
                tsl = slice(tq * 512, (tq + 1) * 512)
                ssl = slice(si * P, (si + 1) * P)
                stt = ps_st.tile([P, 1024], F32, tag="st", name=f"st{hp}_{tq}_{si}")
                for j in range(2):
                    hsl = slice(j * HS, (j + 1) * HS)
                    nc.tensor.matmul(
                        stt[:, j * 512 : (j + 1) * 512],
                        k2[hp][hsl, ssl],
                        q2[hp][hsl, tsl],
                        start=True,
                        stop=True,
                    )
                return stt

            # prefix: q/k pair 0 (all chunks), pair 1 chunk 0, first half of
            # v, interleaved by xT chunk arrival. The rest of v and pair-1
            # q/k become filler, consumed inside the s-loops (v tile si is
            # consumed several iterations before AV(si) needs it; k1 chunk c
            # lands before block 1's ST reaches s-tile 4c).
            for tch in range(4):
                emit_qk_group(wq_sb, q2, 0, tch)
                emit_qk_group(wk_sb, k2, 0, tch)
                for tt in range(4 * tch, 4 * tch + 2):
                    emit_v_group(tt)
            emit_qk_group(wq_sb, q2, 1, 0)
            emit_qk_group(wk_sb, k2, 1, 0)

            # filler work: (fn, args) pairs consumed one per two s-iterations
            filler = [(emit_v_group, (tt,)) for tt in (2, 3, 6, 7, 10, 11, 14, 15)]
            filler += [(emit_qk_group, (wk_sb, k2, 1, tch)) for tch in (1, 2, 3)]
            filler += [(emit_qk_group, (wq_sb, q2, 1, tch)) for tch in (1, 2, 3)]

            # software pipeline: issue ST(si+1) on the PE BEFORE the AV(si)
            # matmuls. AV(si) stalls the in-order PE queue on exp(si); with
            # ST(si+1) ahead of it, the array computes the next scores under
            # the ScalarE exp. The next BLOCK's ST(0) is likewise hoisted
            # into the current block's last iteration.
            blocks = [(tq, hp) for tq in range(4) for hp in range(2)]
            stt = emit_st(blocks[0][1], blocks[0][0], 0)
            for bi, (tq, hp) in enumerate(blocks):
                tsl = slice(tq * 512, (tq + 1) * 512)
                av = [
                    ps_av.tile([HS + 1, 512], F32, tag="av", name=f"av{hp}_{tq}_{j}")
                    for j in range(2)
                ]
                for si in range(16):
                    att = attp.tile([P, 1024], F16, tag="att")
                    nc.scalar.activation(att, stt, func=EXP, scale=float(HS**-0.5))
                    if si < 15:
                        stt = emit_st(hp, tq, si + 1)
                    elif bi + 1 < len(blocks):
                        ntq, nhp = blocks[bi + 1]
                        stt = emit_st(nhp, ntq, 0)
                    if filler and si % 2 == 0:
                        fn, args = filler.pop(0)
                        fn(*args)
                    v3 = v_sb[si].rearrange("p (h e) -> p h e", h=HPC)
                    for j in range(2):
                        nc.tensor.matmul(
                            av[j],
                            v3[:, 2 * hp + j, :],
                            att[:, j * 512 : (j + 1) * 512],
                            start=(si == 0),
                            stop=(si == 15),
                        )
                # normalization is deferred as filler into the next block's
                # s-loop: its Ln/Exp then slot into the ACT stream without
                # stalling it (the in-order ACT queue would otherwise idle
                # waiting for AV(15))
                filler.append((emit_norm, (av, hp, tq)))
                # and after the pair's norms: the t-chunk's projection
                if hp == 1:
                    filler.extend(
                        (emit_proj, (tt,)) for tt in range(4 * tq, 4 * tq + 4)
                    )
            # drain any remaining filler (last block's norm + projection)
            for fn, args in filler:
                fn(*args)


_NC_CACHE = None


def _combined_act_set_id() -> int:
    """Index (into act_info.json act_func_sets) of a set with exp AND ln."""
    try:
        import glob as _glob
        import json as _json
        import neuronxcc

        pat = os.path.join(
            os.path.dirname(neuronxcc.__file__), "pwp", "*", "act_info.json"
        )
        for p in sorted(_glob.glob(pat)):
            sets = _json.load(open(p))["act_func_sets"]
            for i, s in enumerate(sets):
                fns = s.get("act", {})
                if "exp" in fns and "ln" in fns:
                    return i
    except Exception:
        pass
    return 6  # natural_log_exp_and_others in the TRN2 act_info.json


def _dedupe_act_table_loads(nc):
    """Keep one ACT table load (the combined exp+ln set); drop the rest.

    Bacc's insert_act_table_loads assigns exp and ln to different sets and
    thrashes (~2.7us per reload, once per normalization block). Every
    activation we emit (Exp, Ln) lives in the combined set, so a single load
    up front is sufficient.
    """
    set_id = _combined_act_set_id()
    first = True
    for b in nc.m.functions[0].blocks:
        keep = []
        for inst in b.instructions:
            if isinstance(inst, mybir.InstLoadActFuncSet):
                if first:
                    inst.act_func_set_id = set_id
                    first = False
                    keep.append(inst)
            else:
                keep.append(inst)
        b.instructions[:] = keep


def _build():
    global _NC_CACHE
    if _NC_CACHE is not None:
        return _NC_CACHE
    nc = bacc.Bacc("TRN2", target_bir_lowering=False, debug=False, num_devices=NCORES)
    xT = nc.dram_tensor("xT", [D, T], F16, kind="ExternalInput").ap()
    wq = nc.dram_tensor("wq", [D, HPC * HS], F16, kind="ExternalInput").ap()
    wk = nc.dram_tensor("wk", [D, HPC * HS], F16, kind="ExternalInput").ap()
    wv = nc.dram_tensor("wv", [D, HPC * HS], F16, kind="ExternalInput").ap()
    wo = nc.dram_tensor("wo", [HPC * HS, D], F16, kind="ExternalInput").ap()
    bo2 = nc.dram_tensor("bo2", [1, D], F32, kind="ExternalInput").ap()
    y = nc.dram_tensor("y", [T, D], F32, kind="ExternalOutput").ap()
    with tile.TileContext(nc) as tc:
        _emit(tc, xT, wq, wk, wv, wo, bo2, y)
    nc.compile()
    _dedupe_act_table_loads(nc)
    _NC_CACHE = nc
    return nc


def _prep_in_maps(x, Wq, Wk, Wv, Wo, bo):
    x = np.asarray(x, dtype=np.float32)
    Wq = np.asarray(Wq, dtype=np.float32)
    Wk = np.asarray(Wk, dtype=np.float32)
    Wv = np.asarray(Wv, dtype=np.float32)
    Wo = np.asarray(Wo, dtype=np.float32)
    bo = np.asarray(bo, dtype=np.float32)
    in_maps = []
    for c in range(NCORES):
        b, hh = divmod(c, 2)
        hb = hh * HPC
        in_maps.append(
            {
                "xT": to_fp16(x[b].T),
                "wq": to_fp16(Wq[hb : hb + HPC].transpose(1, 0, 2).reshape(D, HPC * HS)),
                "wk": to_fp16(Wk[hb : hb + HPC].transpose(1, 0, 2).reshape(D, HPC * HS)),
                "wv": to_fp16(Wv[hb : hb + HPC].transpose(1, 0, 2).reshape(D, HPC * HS)),
                "wo": to_fp16(Wo[hb * HS : (hb + HPC) * HS, :]),
                "bo2": np.ascontiguousarray((0.5 * bo).reshape(1, D).astype(np.float32)),
            }
        )
    return in_maps


def _run(in_maps, trace=False):
    nc = _build()
    return run_bass_kernel_spmd(nc, in_maps, list(range(NCORES)), trace=trace)


def _run_prof(in_maps, tmpdir):
    nc = _build()
    return run_bass_kernel_spmd(
        nc, in_maps, list(range(NCORES)), trace=True, tmpdir=tmpdir
    )


def kernel(x, Wq, Wk, Wv, Wo, bo):
    in_maps = _prep_in_maps(x, Wq, Wk, Wv, Wo, bo)
    res = _run(in_maps)
    y = np.empty((B, T, D), dtype=np.float32)
    for b in range(B):
        y[b] = res.results[2 * b]["y"] + res.results[2 * b + 1]["y"]
    return y



# revision 20
# speedup vs baseline: 1.1147x; 1.1147x over previous
"""Multi-head attention forward on 8 Trainium2 NeuronCores (Bass/Tile).

Problem: B=4, T=2048, D=512, H=8, HS=64, fp32.
  q/k/v = einsum('btd,hde->bhte', x, W{q,k,v})
  att   = softmax(q k^T / sqrt(HS))
  out   = (att v) concat-heads @ Wo + bo

Sharding (8 cores): core c -> batch b=c//2, heads hb=4*(c%2)..hb+4
(data parallel on B x tensor parallel on H). Each core computes its 4 heads'
attention and a partial output projection against its 256 rows of Wo (bias
halved per core); the host sums the two partials per batch.

On-device dataflow per core (all matmuls in float16: 1 cycle/row on PE,
~1e-3 rel err; PSUM accumulation is fp32):
  phase 1: qT/kT per head-pair [128=2*HS, T] and v [T, 4*(HS+1)] (ones column
           appended per head for the softmax denominator) from xT [D, T].
  phase 2: per head-pair, per 512-wide t-chunk, loop s-tiles of 128:
           ST[s,t] matmuls (K=HS=64, two heads row-packed at partitions 0/64),
           one ScalarE exp over [128, 1024] PSUM (scale=1/8 folded in),
           AV matmuls accumulate [65, 512] (row 64 = sum of exp).
           Then normalize: DVE fast-reciprocal of row 64, partition-broadcast,
           multiply -> outT [4*HS, T].
  phase 3: y[t,:] = outT.T @ Wo_rows + 0.5*bo via accumulating matmuls,
           DVE bias add, DMA out.

The ScalarE exp stream is the critical path (128 x ~1.1us); the emission
schedule keeps it saturated: a minimal prologue (only q/k chunk 0 of pair 0
before the first score tile), all other projection work hand-placed into
specific s-iterations as PE filler with per-tile deadlines, and the
normalization entirely off ScalarE (DVE reciprocal instead of Ln/Exp).
"""
import os
import sys

sys.path.insert(0, "/opt/trn_rl_repo")

import numpy as np
from contextlib import ExitStack

import concourse.bacc as bacc
import concourse.tile as tile
from concourse import mybir
from concourse.bass_utils import run_bass_kernel_spmd

B, T, D, H, HS = 4, 2048, 512, 8, 64
NCORES = 8
P = 128
HPC = 4  # heads per core
F32 = mybir.dt.float32
F16 = mybir.dt.float16
EXP = mybir.ActivationFunctionType.Exp


def to_fp16(x: np.ndarray) -> np.ndarray:
    return np.ascontiguousarray(np.asarray(x, dtype=np.float32).astype(np.float16))


def _emit(tc, xT, wq, wk, wv, wo, bo2, y):
    nc = tc.nc
    with ExitStack() as ctx:
        persist = ctx.enter_context(tc.tile_pool(name="persist", bufs=1))

        # ---- persistent SBUF tiles ----
        xt_sb = [persist.tile([P, T], F16, tag=f"xt{i}", name=f"xt{i}") for i in range(4)]
        wq_sb = [persist.tile([P, 2 * P], F16, tag=f"wq{i}", name=f"wq{i}") for i in range(4)]
        wk_sb = [persist.tile([P, 2 * P], F16, tag=f"wk{i}", name=f"wk{i}") for i in range(4)]
        wv_sb = [persist.tile([P, 2 * P], F16, tag=f"wv{i}", name=f"wv{i}") for i in range(4)]
        wo_sb = [persist.tile([P, D], F16, tag=f"wo{i}", name=f"wo{i}") for i in range(2)]
        bo2_sb = persist.tile([1, D], F32, tag="bo2")
        bo_bc = persist.tile([P, D], F32, tag="bo_bc")
        q2 = [persist.tile([P, T], F16, tag=f"q2{i}", name=f"q2_{i}") for i in range(2)]
        k2 = [persist.tile([P, T], F16, tag=f"k2{i}", name=f"k2_{i}") for i in range(2)]
        v_sb = [persist.tile([P, HPC * (HS + 1)], F16, tag=f"v{i}", name=f"v{i}") for i in range(16)]
        out2 = [persist.tile([P, T], F16, tag=f"o2{i}", name=f"o2_{i}") for i in range(2)]
        ones_v16 = persist.tile([P, HPC], F16, tag="ones_v16")
        warm_in = persist.tile([P, 512], F16, tag="warm_in")

        # PE warm-up: ~3.5us of dummy matmuls during the DMA lead-in flips
        # the HAM clock gate to 2.4 GHz before the real matmuls start.
        with tc.tile_pool(name="ps_warm", bufs=1, space="PSUM") as ps_warm:
            nc.vector.memset(warm_in, 0.5)
            wp = ps_warm.tile([P, 512], F32, tag="warm")
            for _ in range(8):
                nc.tensor.matmul(wp, warm_in[:, 0:P], warm_in, start=True, stop=True)

        # Input DMAs on the three DMA-capable queues (sync/scalar/gpsimd).
        # The four chunk-0 xT tiles lead each queue so the first projection
        # group can start ~2.5us after issue; everything else is ordered by
        # its first consumer's deadline. The scalar queue is kept short so
        # the ACT sequencer is free before the first exp.
        c0 = slice(0, 512)

        def dsl(i):
            return slice(i * P, (i + 1) * P)

        # Strict deadline order: the 12 tiles gating the first q/k groups
        # (xT chunk 0, wq, wk) are the first issues on every queue so their
        # transfers don't queue behind the bulk xT chunks on HBM.
        nc.sync.dma_start(out=xt_sb[0][:, c0], in_=xT[dsl(0), c0])
        nc.scalar.dma_start(out=xt_sb[2][:, c0], in_=xT[dsl(2), c0])
        nc.gpsimd.dma_start(out=xt_sb[3][:, c0], in_=xT[dsl(3), c0])
        nc.sync.dma_start(out=xt_sb[1][:, c0], in_=xT[dsl(1), c0])
        nc.scalar.dma_start(out=wq_sb[0], in_=wq[dsl(0), :])
        nc.scalar.dma_start(out=wq_sb[1], in_=wq[dsl(1), :])
        nc.sync.dma_start(out=wq_sb[2], in_=wq[dsl(2), :])
        nc.sync.dma_start(out=wq_sb[3], in_=wq[dsl(3), :])
        for i in range(4):
            nc.gpsimd.dma_start(out=wk_sb[i], in_=wk[dsl(i), :])
        nc.sync.dma_start(out=wv_sb[0], in_=wv[dsl(0), :])
        nc.sync.dma_start(out=wv_sb[1], in_=wv[dsl(1), :])
        nc.gpsimd.dma_start(out=wv_sb[2], in_=wv[dsl(2), :])
        nc.gpsimd.dma_start(out=wv_sb[3], in_=wv[dsl(3), :])
        for tch in range(1, 4):
            csl = slice(tch * 512, (tch + 1) * 512)
            for i in range(4):
                nc.sync.dma_start(out=xt_sb[i][:, csl], in_=xT[dsl(i), csl])
        for i in range(2):
            nc.gpsimd.dma_start(out=wo_sb[i], in_=wo[i * P : (i + 1) * P, :])
        nc.gpsimd.dma_start(out=bo2_sb, in_=bo2)
        nc.gpsimd.partition_broadcast(bo_bc, bo2_sb)
        nc.vector.memset(ones_v16, 1.0)

        # One shared PSUM layout: st 2x[128,1024] (4 banks) + 4 general
        # [128,512] slots (tag "av": projection groups AND the AV
        # accumulators) = 8 banks.
        with (
            tc.tile_pool(name="ps_st", bufs=2, space="PSUM") as ps_st,
            tc.tile_pool(name="ps_av", bufs=4, space="PSUM") as ps_av,
            tc.tile_pool(name="attp", bufs=6) as attp,
            tc.tile_pool(name="nrm", bufs=6) as nrm,
            tc.tile_pool(name="yout", bufs=3) as yout,
        ):
            blocks = [(tq, hp) for tq in range(4) for hp in range(2)]
            avs = [None] * len(blocks)
            # Explicit cross-engine deps where Tile's subtile tracking is
            # unreliable (partition-subrange reads of DVE-written tiles):
            # q/k chunk copies -> ST matmuls, norm multiplies -> projection.
            qk_cp = {}
            nrm_mul = {}
            av_last = {}
            v_cp = {}

            def emit_qk_group(kind, pr, tch):
                # one [128,512] chunk of the q or k projection (4 accumulating
                # matmuls over D, then a cast-copy to fp16 SBUF)
                w_sb, dst = (wq_sb, q2) if kind == "q" else (wk_sb, k2)
                psl = slice(pr * P, (pr + 1) * P)
                tsl = slice(tch * 512, (tch + 1) * 512)
                pt = ps_av.tile(
                    [P, 512], F32, tag="av", name=f"qk{kind}_{pr}_{tch}"
                )
                for di in range(4):
                    nc.tensor.matmul(
                        pt,
                        w_sb[di][:, psl],
                        xt_sb[di][:, tsl],
                        start=(di == 0),
                        stop=(di == 3),
                    )
                qk_cp[(kind, pr, tch)] = nc.vector.tensor_copy(dst[pr][:, tsl], pt)

            def emit_v_group(tt):
                ttsl = slice(tt * P, (tt + 1) * P)
                pv = ps_av.tile([P, 2 * P], F32, tag="av", name=f"pv{tt}")
                for di in range(4):
                    nc.tensor.matmul(
                        pv,
                        xt_sb[di][:, ttsl],
                        wv_sb[di],
                        start=(di == 0),
                        stop=(di == 3),
                    )
                v3 = v_sb[tt].rearrange("p (h e) -> p h e", h=HPC)
                c1 = nc.vector.tensor_copy(
                    v3[:, :, 0:HS], pv.rearrange("p (h e) -> p h e", h=HPC)
                )
                c2 = nc.vector.tensor_copy(v3[:, :, HS], ones_v16)
                v_cp[tt] = (c1, c2)

            def emit_proj(tt):
                # output projection t-tile + bias add (from broadcast bo_bc)
                ttsl = slice(tt * P, (tt + 1) * P)
                tq = tt // 4
                yp = ps_av.tile([P, D], F32, tag="av", name=f"yp{tt}")
                for hp in range(2):
                    mm = nc.tensor.matmul(
                        yp, out2[hp][:, ttsl], wo_sb[hp], start=(hp == 0), stop=(hp == 1)
                    )
                    for j in range(2):
                        dep = nrm_mul.get((hp, tq, j))
                        if dep is not None:
                            tile.add_dep_helper(mm.ins, dep.ins, reason="norm ready")
                ys = yout.tile([P, D], F16, tag="y")
                nc.vector.tensor_add(ys, yp, bo_bc)
                nc.sync.dma_start(out=y[ttsl, :], in_=ys)

            def emit_norm(b, tail=False):
                # divide the pair's unnormalized outputs by their sums of
                # exp: DVE fast reciprocal of each sum row (partition 64 of
                # the accumulator), partition-broadcast on GpSimd, multiply
                # on DVE. No ScalarE involvement - it is the critical engine.
                tq, hp = blocks[b]
                av = avs[b]
                tsl = slice(tq * 512, (tq + 1) * 512)
                prev_ri = None
                for j in range(2):
                    # Stage the sum row to partition 0 first: the custom DVE
                    # reciprocal does not honor a nonzero partition offset on
                    # its input AP. Tile also misses the partition-subrange
                    # read of the PSUM accumulator, so tie the reads to the
                    # block's final AV matmul explicitly.
                    fin = av_last[(b, j)]
                    den = nrm.tile([1, 512], F32, tag="den", name=f"den{b}_{j}")
                    ci = nc.vector.tensor_copy(den, av[j][HS : HS + 1, :])
                    tile.add_dep_helper(ci.ins, fin.ins, reason="av accum done")
                    if tail and prev_ri is not None:
                        # in the drain the scheduler otherwise interleaves
                        # the j chains (copy,copy,recip,recip), delaying the
                        # first broadcast; force j0's chain to finish first
                        tile.add_dep_helper(ci.ins, prev_ri.ins, reason="tail order")
                    rc = nrm.tile([1, 512], F32, tag="rc", name=f"rc{b}_{j}")
                    ri = nc.vector.reciprocal_approx_fast(rc, den)
                    prev_ri = ri
                    bco = nrm.tile([HS, 512], F32, tag="bco", name=f"bco{b}_{j}")
                    nc.gpsimd.partition_broadcast(bco, rc)
                    mi = nc.vector.tensor_mul(
                        out2[hp][j * HS : (j + 1) * HS, tsl],
                        av[j][0:HS, :],
                        bco,
                    )
                    tile.add_dep_helper(mi.ins, fin.ins, reason="av accum done")
                    nrm_mul[(hp, tq, j)] = mi

            def emit_st(hp, tq, si):
                # scores for both heads of the pair, row-packed at
                # partitions 0 / 64 (K=64 each)
                tsl = slice(tq * 512, (tq + 1) * 512)
                ssl = slice(si * P, (si + 1) * P)
                stt = ps_st.tile([P, 1024], F32, tag="st", name=f"st{hp}_{tq}_{si}")
                for j in range(2):
                    hsl = slice(j * HS, (j + 1) * HS)
                    mm = nc.tensor.matmul(
                        stt[:, j * 512 : (j + 1) * 512],
                        k2[hp][hsl, ssl],
                        q2[hp][hsl, tsl],
                        start=True,
                        stop=True,
                    )
                    for key in (("q", hp, tq), ("k", hp, si // 4)):
                        cp = qk_cp.get(key)
                        if cp is not None:
                            tile.add_dep_helper(mm.ins, cp.ins, reason="qk chunk")
                return stt

            # Filler schedule: sched[b][si] = list of thunks issued on the PE
            # (or DVE/GpSimd for norms) inside iteration si of block b,
            # between the hoisted ST(si+1) and the AV(si) matmuls. Every item
            # is placed ahead of its consumer's deadline:
            #   k chunk c of the running pair  -> before ST(4c) issues (iter 4c-1)
            #   v tile si                      -> before AV(si) (iter si)
            #   q/k chunk 0 of the next pair   -> before the next block's ST(0)
            #   q chunk tq                     -> before block (tq,hp) starts
            #   norm of block b-1              -> iter 0 (frees its PSUM pair)
            #   proj tiles of t-chunk tq       -> after both norms of tq
            sched = [dict() for _ in range(len(blocks))]

            def put(b, it, fn, *args):
                sched[b].setdefault(it, []).append((fn, args))

            # block 0: v tiles + remaining k chunks of pair 0, then pair 1
            put(0, 0, emit_v_group, 0)
            put(0, 0, emit_v_group, 1)
            put(0, 0, emit_qk_group, "k", 0, 1)
            put(0, 1, emit_v_group, 2)
            put(0, 1, emit_v_group, 3)
            put(0, 2, emit_v_group, 4)
            put(0, 3, emit_qk_group, "k", 0, 2)
            put(0, 3, emit_v_group, 5)
            put(0, 4, emit_v_group, 6)
            put(0, 4, emit_v_group, 7)
            put(0, 5, emit_v_group, 8)
            put(0, 6, emit_v_group, 9)
            put(0, 7, emit_qk_group, "k", 0, 3)
            put(0, 7, emit_v_group, 10)
            put(0, 8, emit_v_group, 11)
            put(0, 9, emit_v_group, 12)
            put(0, 10, emit_v_group, 13)
            put(0, 11, emit_qk_group, "q", 1, 0)
            put(0, 12, emit_v_group, 14)
            put(0, 13, emit_qk_group, "k", 1, 0)
            put(0, 14, emit_v_group, 15)
            # block 1: rest of pair-1 k, q chunk for block 2
            put(1, 0, emit_norm, 0)
            put(1, 2, emit_qk_group, "k", 1, 1)
            put(1, 5, emit_qk_group, "k", 1, 2)
            put(1, 9, emit_qk_group, "k", 1, 3)
            put(1, 12, emit_qk_group, "q", 0, 1)
            # blocks 2..7: norms, projections (3+1 split keeps PE slack
            # even), and the remaining q chunks one block ahead of use
            # proj goes at iter >= 5: its explicit dep on the norm multiply
            # (which completes ~3 iterations after the block starts) would
            # otherwise head-of-line-block the in-order PE queue and stall
            # the exp stream at every block boundary.
            put(2, 0, emit_norm, 1)
            put(2, 5, emit_proj, 0)
            put(2, 8, emit_proj, 1)
            put(2, 11, emit_proj, 2)
            put(2, 13, emit_qk_group, "q", 1, 1)
            put(3, 0, emit_norm, 2)
            put(3, 5, emit_proj, 3)
            put(3, 8, emit_qk_group, "q", 0, 2)
            put(4, 0, emit_norm, 3)
            put(4, 5, emit_proj, 4)
            put(4, 8, emit_proj, 5)
            put(4, 11, emit_proj, 6)
            put(4, 13, emit_qk_group, "q", 1, 2)
            put(5, 0, emit_norm, 4)
            put(5, 5, emit_proj, 7)
            put(5, 8, emit_qk_group, "q", 0, 3)
            put(6, 0, emit_norm, 5)
            put(6, 5, emit_proj, 8)
            put(6, 8, emit_proj, 9)
            put(6, 11, emit_proj, 10)
            put(6, 13, emit_qk_group, "q", 1, 3)
            put(7, 0, emit_norm, 6)
            put(7, 5, emit_proj, 11)

            # Minimal prologue: only the chunk-0 q/k of pair 0, then straight
            # into the first score tile so the exp stream starts ~17us in.
            emit_qk_group("q", 0, 0)
            emit_qk_group("k", 0, 0)

            # software pipeline: issue ST(si+1) on the PE BEFORE the AV(si)
            # matmuls. AV(si) stalls the in-order PE queue on exp(si); with
            # ST(si+1) ahead of it, the array computes the next scores under
            # the ScalarE exp. The next BLOCK's ST(0) is likewise hoisted
            # into the current block's last iteration.
            stt = emit_st(blocks[0][1], blocks[0][0], 0)
            for bi, (tq, hp) in enumerate(blocks):
                tsl = slice(tq * 512, (tq + 1) * 512)
                av = [
                    ps_av.tile([HS + 1, 512], F32, tag="av", name=f"av{hp}_{tq}_{j}")
                    for j in range(2)
                ]
                avs[bi] = av
                for si in range(16):
                    att = attp.tile([P, 1024], F16, tag="att")
                    nc.scalar.activation(att, stt, func=EXP, scale=float(HS**-0.5))
                    if si < 15:
                        stt = emit_st(hp, tq, si + 1)
                    elif bi + 1 < len(blocks):
                        ntq, nhp = blocks[bi + 1]
                        stt = emit_st(nhp, ntq, 0)
                    for fn, args in sched[bi].get(si, ()):
                        fn(*args)
                    v3 = v_sb[si].rearrange("p (h e) -> p h e", h=HPC)
                    for j in range(2):
                        mm_av = nc.tensor.matmul(
                            av[j],
                            v3[:, 2 * hp + j, :],
                            att[:, j * 512 : (j + 1) * 512],
                            start=(si == 0),
                            stop=(si == 15),
                        )
                        for cp in v_cp.get(si, ()):
                            tile.add_dep_helper(mm_av.ins, cp.ins, reason="v tile")
                        if si == 15:
                            av_last[(bi, j)] = mm_av
            # drain: last block's norm + the final t-chunk's projection.
            # A few dummy matmuls keep the PE out of its low p-state while
            # the cross-engine norm chain runs, so the projection matmuls
            # execute at full clock.
            warm2 = ps_av.tile([P, 512], F32, tag="av", name="warm2")
            for _ in range(5):
                nc.tensor.matmul(warm2, warm_in[:, 0:P], warm_in, start=True, stop=True)
            emit_norm(7, tail=True)
            for tt in range(12, 16):
                emit_proj(tt)


_NC_CACHE = None


def _combined_act_set_id() -> int:
    """Index (into act_info.json act_func_sets) of a set with exp."""
    try:
        import glob as _glob
        import json as _json
        import neuronxcc

        pat = os.path.join(
            os.path.dirname(neuronxcc.__file__), "pwp", "*", "act_info.json"
        )
        for p in sorted(_glob.glob(pat)):
            sets = _json.load(open(p))["act_func_sets"]
            for i, s in enumerate(sets):
                fns = s.get("act", {})
                if "exp" in fns and "ln" in fns:
                    return i
    except Exception:
        pass
    return 6  # natural_log_exp_and_others in the TRN2 act_info.json


def _dedupe_act_table_loads(nc):
    """Keep one ACT table load; drop the rest.

    Bacc's insert_act_table_loads can thrash (~2.7us per reload). Every
    activation we emit (Exp) lives in the combined set, so a single load
    up front is sufficient.
    """
    set_id = _combined_act_set_id()
    first = True
    for b in nc.m.functions[0].blocks:
        keep = []
        for inst in b.instructions:
            if isinstance(inst, mybir.InstLoadActFuncSet):
                if first:
                    inst.act_func_set_id = set_id
                    first = False
                    keep.append(inst)
            else:
                keep.append(inst)
        b.instructions[:] = keep


def _build():
    global _NC_CACHE
    if _NC_CACHE is not None:
        return _NC_CACHE
    nc = bacc.Bacc("TRN2", target_bir_lowering=False, debug=False, num_devices=NCORES)
    xT = nc.dram_tensor("xT", [D, T], F16, kind="ExternalInput").ap()
    wq = nc.dram_tensor("wq", [D, HPC * HS], F16, kind="ExternalInput").ap()
    wk = nc.dram_tensor("wk", [D, HPC * HS], F16, kind="ExternalInput").ap()
    wv = nc.dram_tensor("wv", [D, HPC * HS], F16, kind="ExternalInput").ap()
    wo = nc.dram_tensor("wo", [HPC * HS, D], F16, kind="ExternalInput").ap()
    bo2 = nc.dram_tensor("bo2", [1, D], F32, kind="ExternalInput").ap()
    y = nc.dram_tensor("y", [T, D], F16, kind="ExternalOutput").ap()
    with tile.TileContext(nc) as tc:
        _emit(tc, xT, wq, wk, wv, wo, bo2, y)
    nc.compile()
    _dedupe_act_table_loads(nc)
    _NC_CACHE = nc
    return nc


def _prep_in_maps(x, Wq, Wk, Wv, Wo, bo):
    x = np.asarray(x, dtype=np.float32)
    Wq = np.asarray(Wq, dtype=np.float32)
    Wk = np.asarray(Wk, dtype=np.float32)
    Wv = np.asarray(Wv, dtype=np.float32)
    Wo = np.asarray(Wo, dtype=np.float32)
    bo = np.asarray(bo, dtype=np.float32)
    in_maps = []
    for c in range(NCORES):
        b, hh = divmod(c, 2)
        hb = hh * HPC
        in_maps.append(
            {
                "xT": to_fp16(x[b].T),
                "wq": to_fp16(Wq[hb : hb + HPC].transpose(1, 0, 2).reshape(D, HPC * HS)),
                "wk": to_fp16(Wk[hb : hb + HPC].transpose(1, 0, 2).reshape(D, HPC * HS)),
                "wv": to_fp16(Wv[hb : hb + HPC].transpose(1, 0, 2).reshape(D, HPC * HS)),
                "wo": to_fp16(Wo[hb * HS : (hb + HPC) * HS, :]),
                "bo2": np.ascontiguousarray((0.5 * bo).reshape(1, D).astype(np.float32)),
            }
        )
    return in_maps


def _run(in_maps, trace=False):
    nc = _build()
    return run_bass_kernel_spmd(nc, in_maps, list(range(NCORES)), trace=trace)


def _run_prof(in_maps, tmpdir):
    nc = _build()
    return run_bass_kernel_spmd(
        nc, in_maps, list(range(NCORES)), trace=True, tmpdir=tmpdir
    )


def kernel(x, Wq, Wk, Wv, Wo, bo):
    in_maps = _prep_in_maps(x, Wq, Wk, Wv, Wo, bo)
    res = _run(in_maps)
    y = np.empty((B, T, D), dtype=np.float32)
    for b in range(B):
        y[b] = res.results[2 * b]["y"].astype(np.float32) + res.results[
            2 * b + 1
        ]["y"].astype(np.float32)
    return y


# revision 24
# speedup vs baseline: 1.1186x; 1.0035x over previous
"""Multi-head attention forward on 8 Trainium2 NeuronCores (Bass/Tile).

Problem: B=4, T=2048, D=512, H=8, HS=64, fp32.
  q/k/v = einsum('btd,hde->bhte', x, W{q,k,v})
  att   = softmax(q k^T / sqrt(HS))
  out   = (att v) concat-heads @ Wo + bo

Sharding (8 cores): core c -> batch b=c//2, heads hb=4*(c%2)..hb+4
(data parallel on B x tensor parallel on H). Each core computes its 4 heads'
attention and a partial output projection against its 256 rows of Wo (bias
halved per core); the host sums the two partials per batch.

On-device dataflow per core (all matmuls in float16: 1 cycle/row on PE,
~1e-3 rel err; PSUM accumulation is fp32):
  phase 1: qT/kT per head-pair [128=2*HS, T] and v [T, 4*(HS+1)] (ones column
           appended per head for the softmax denominator) from xT [D, T].
  phase 2: per head-pair, per 512-wide t-chunk, loop s-tiles of 128:
           ST[s,t] matmuls (K=HS=64, two heads row-packed at partitions 0/64),
           one ScalarE exp over [128, 1024] PSUM (scale=1/8 folded in),
           AV matmuls accumulate [65, 512] (row 64 = sum of exp).
           Then normalize: DVE fast-reciprocal of row 64, partition-broadcast,
           multiply -> outT [4*HS, T].
  phase 3: y[t,:] = outT.T @ Wo_rows + 0.5*bo via accumulating matmuls,
           DVE bias add, DMA out.

The ScalarE exp stream is the critical path (128 x ~1.1us); the emission
schedule keeps it saturated: a minimal prologue (only q/k chunk 0 of pair 0
before the first score tile), all other projection work hand-placed into
specific s-iterations as PE filler with per-tile deadlines, and the
normalization entirely off ScalarE (DVE reciprocal instead of Ln/Exp).
"""
import os
import sys

sys.path.insert(0, "/opt/trn_rl_repo")

import numpy as np
from contextlib import ExitStack

import concourse.bacc as bacc
import concourse.tile as tile
from concourse import mybir
from concourse.bass_utils import run_bass_kernel_spmd

B, T, D, H, HS = 4, 2048, 512, 8, 64
NCORES = 8
P = 128
HPC = 4  # heads per core
F32 = mybir.dt.float32
F16 = mybir.dt.float16
EXP = mybir.ActivationFunctionType.Exp


def to_fp16(x: np.ndarray) -> np.ndarray:
    return np.ascontiguousarray(np.asarray(x, dtype=np.float32).astype(np.float16))


def _emit(tc, xT, wq, wk, wv, wo, bo2, y):
    nc = tc.nc
    with ExitStack() as ctx:
        persist = ctx.enter_context(tc.tile_pool(name="persist", bufs=1))

        # ---- persistent SBUF tiles ----
        xt_sb = [persist.tile([P, T], F16, tag=f"xt{i}", name=f"xt{i}") for i in range(4)]
        wq_sb = [persist.tile([P, 2 * P], F16, tag=f"wq{i}", name=f"wq{i}") for i in range(4)]
        wk_sb = [persist.tile([P, 2 * P], F16, tag=f"wk{i}", name=f"wk{i}") for i in range(4)]
        wv_sb = [persist.tile([P, 2 * P], F16, tag=f"wv{i}", name=f"wv{i}") for i in range(4)]
        wo_sb = [persist.tile([P, D], F16, tag=f"wo{i}", name=f"wo{i}") for i in range(2)]
        bo2_sb = persist.tile([1, D], F32, tag="bo2")
        bo_bc = persist.tile([P, D], F32, tag="bo_bc")
        q2 = [persist.tile([P, T], F16, tag=f"q2{i}", name=f"q2_{i}") for i in range(2)]
        k2 = [persist.tile([P, T], F16, tag=f"k2{i}", name=f"k2_{i}") for i in range(2)]
        v_sb = [persist.tile([P, HPC * 2 * HS], F16, tag=f"v{i}", name=f"v{i}") for i in range(16)]
        out2 = [persist.tile([P, T], F16, tag=f"o2{i}", name=f"o2_{i}") for i in range(2)]
        ones_v16 = persist.tile([P, HPC], F16, tag="ones_v16")
        warm_in = persist.tile([P, 512], F16, tag="warm_in")

        # PE warm-up: ~3.5us of dummy matmuls during the DMA lead-in flips
        # the HAM clock gate to 2.4 GHz before the real matmuls start.
        with tc.tile_pool(name="ps_warm", bufs=1, space="PSUM") as ps_warm:
            nc.vector.memset(warm_in, 0.5)
            wp = ps_warm.tile([P, 512], F32, tag="warm")
            for _ in range(8):
                nc.tensor.matmul(wp, warm_in[:, 0:P], warm_in, start=True, stop=True)

        # Input DMAs on the three DMA-capable queues (sync/scalar/gpsimd).
        # The four chunk-0 xT tiles lead each queue so the first projection
        # group can start ~2.5us after issue; everything else is ordered by
        # its first consumer's deadline. The scalar queue is kept short so
        # the ACT sequencer is free before the first exp.
        c0 = slice(0, 512)

        def dsl(i):
            return slice(i * P, (i + 1) * P)

        # Strict deadline order: the 12 tiles gating the first q/k groups
        # (xT chunk 0, wq, wk) are the first issues on every queue so their
        # transfers don't queue behind the bulk xT chunks on HBM.
        nc.sync.dma_start(out=xt_sb[0][:, c0], in_=xT[dsl(0), c0])
        nc.scalar.dma_start(out=xt_sb[2][:, c0], in_=xT[dsl(2), c0])
        nc.gpsimd.dma_start(out=xt_sb[3][:, c0], in_=xT[dsl(3), c0])
        nc.sync.dma_start(out=xt_sb[1][:, c0], in_=xT[dsl(1), c0])
        nc.scalar.dma_start(out=wq_sb[0], in_=wq[dsl(0), :])
        nc.scalar.dma_start(out=wq_sb[1], in_=wq[dsl(1), :])
        nc.sync.dma_start(out=wq_sb[2], in_=wq[dsl(2), :])
        nc.sync.dma_start(out=wq_sb[3], in_=wq[dsl(3), :])
        for i in range(4):
            nc.gpsimd.dma_start(out=wk_sb[i], in_=wk[dsl(i), :])
        nc.sync.dma_start(out=wv_sb[0], in_=wv[dsl(0), :])
        nc.sync.dma_start(out=wv_sb[1], in_=wv[dsl(1), :])
        nc.gpsimd.dma_start(out=wv_sb[2], in_=wv[dsl(2), :])
        nc.gpsimd.dma_start(out=wv_sb[3], in_=wv[dsl(3), :])
        for tch in range(1, 4):
            csl = slice(tch * 512, (tch + 1) * 512)
            for i in range(4):
                nc.sync.dma_start(out=xt_sb[i][:, csl], in_=xT[dsl(i), csl])
        for i in range(2):
            nc.gpsimd.dma_start(out=wo_sb[i], in_=wo[i * P : (i + 1) * P, :])
        nc.gpsimd.dma_start(out=bo2_sb, in_=bo2)
        nc.gpsimd.partition_broadcast(bo_bc, bo2_sb)
        nc.vector.memset(ones_v16, 1.0)

        # One shared PSUM layout: st 2x[128,1024] (4 banks) + 4 general
        # [128,512] slots (tag "av": projection groups AND the AV
        # accumulators) = 8 banks.
        with (
            tc.tile_pool(name="ps_st", bufs=2, space="PSUM") as ps_st,
            tc.tile_pool(name="ps_av", bufs=4, space="PSUM") as ps_av,
            tc.tile_pool(name="attp", bufs=6) as attp,
            tc.tile_pool(name="nrm", bufs=6) as nrm,
            tc.tile_pool(name="yout", bufs=3) as yout,
        ):
            blocks = [(tq, hp) for tq in range(4) for hp in range(2)]
            avs = [None] * len(blocks)
            # Explicit cross-engine deps where Tile's subtile tracking is
            # unreliable (partition-subrange reads of DVE-written tiles):
            # q/k chunk copies -> ST matmuls, norm multiplies -> projection.
            qk_cp = {}
            nrm_mul = {}
            av_last = {}
            v_cp = {}

            def emit_qk_group(kind, pr, tch):
                # one [128,512] chunk of the q or k projection (4 accumulating
                # matmuls over D, then a cast-copy to fp16 SBUF)
                w_sb, dst = (wq_sb, q2) if kind == "q" else (wk_sb, k2)
                psl = slice(pr * P, (pr + 1) * P)
                tsl = slice(tch * 512, (tch + 1) * 512)
                pt = ps_av.tile(
                    [P, 512], F32, tag="av", name=f"qk{kind}_{pr}_{tch}"
                )
                for di in range(4):
                    nc.tensor.matmul(
                        pt,
                        w_sb[di][:, psl],
                        xt_sb[di][:, tsl],
                        start=(di == 0),
                        stop=(di == 3),
                    )
                qk_cp[(kind, pr, tch)] = nc.vector.tensor_copy(dst[pr][:, tsl], pt)

            def emit_v_group(tt):
                ttsl = slice(tt * P, (tt + 1) * P)
                pv = ps_av.tile([P, 2 * P], F32, tag="av", name=f"pv{tt}")
                for di in range(4):
                    nc.tensor.matmul(
                        pv,
                        xt_sb[di][:, ttsl],
                        wv_sb[di],
                        start=(di == 0),
                        stop=(di == 3),
                    )
                # Per head, the 128 lhsT columns are [ones, 63 junk, v x 64]:
                # the denominator accumulates on partition 0 of the AV tile
                # (read directly by the DVE reciprocal -- the custom op
                # mishandles nonzero partition offsets) and the numerators
                # land at partition 64 (quadrant-aligned for the multiply).
                # The junk columns cost nothing: matmul time depends only on
                # the moving free size, and their accumulator rows are never
                # read.
                v3 = v_sb[tt].rearrange("p (h e) -> p h e", h=HPC)
                c1 = nc.vector.tensor_copy(
                    v3[:, :, HS : 2 * HS], pv.rearrange("p (h e) -> p h e", h=HPC)
                )
                c2 = nc.vector.tensor_copy(v3[:, :, 0], ones_v16)
                v_cp[tt] = (c1, c2)

            def emit_proj(tt):
                # output projection t-tile + bias add (from broadcast bo_bc)
                ttsl = slice(tt * P, (tt + 1) * P)
                tq = tt // 4
                yp = ps_av.tile([P, D], F32, tag="av", name=f"yp{tt}")
                for hp in range(2):
                    mm = nc.tensor.matmul(
                        yp, out2[hp][:, ttsl], wo_sb[hp], start=(hp == 0), stop=(hp == 1)
                    )
                    for j in range(2):
                        dep = nrm_mul.get((hp, tq, j))
                        if dep is not None:
                            tile.add_dep_helper(mm.ins, dep.ins, reason="norm ready")
                ys = yout.tile([P, D], F16, tag="y")
                nc.vector.tensor_add(ys, yp, bo_bc)
                nc.sync.dma_start(out=y[ttsl, :], in_=ys)

            def emit_norm(b, tail=False):
                # divide the pair's unnormalized outputs by their sums of
                # exp: DVE fast reciprocal of each sum row (partition 64 of
                # the accumulator), partition-broadcast on GpSimd, multiply
                # on DVE. No ScalarE involvement - it is the critical engine.
                tq, hp = blocks[b]
                av = avs[b]
                tsl = slice(tq * 512, (tq + 1) * 512)
                for j in range(2):
                    # Tile misses the partition-subrange read of the PSUM
                    # accumulator, so tie both reads to the block's final AV
                    # matmul explicitly.
                    fin = av_last[(b, j)]
                    rc = nrm.tile([1, 512], F32, tag="rc", name=f"rc{b}_{j}")
                    ri = nc.vector.reciprocal_approx_fast(rc, av[j][0:1, :])
                    tile.add_dep_helper(ri.ins, fin.ins, reason="av accum done")
                    bco = nrm.tile([HS, 512], F32, tag="bco", name=f"bco{b}_{j}")
                    nc.gpsimd.partition_broadcast(bco, rc)
                    mi = nc.vector.tensor_mul(
                        out2[hp][j * HS : (j + 1) * HS, tsl],
                        av[j][HS : 2 * HS, :],
                        bco,
                    )
                    tile.add_dep_helper(mi.ins, fin.ins, reason="av accum done")
                    nrm_mul[(hp, tq, j)] = mi

            def emit_st(hp, tq, si):
                # scores for both heads of the pair, row-packed at
                # partitions 0 / 64 (K=64 each)
                tsl = slice(tq * 512, (tq + 1) * 512)
                ssl = slice(si * P, (si + 1) * P)
                stt = ps_st.tile([P, 1024], F32, tag="st", name=f"st{hp}_{tq}_{si}")
                for j in range(2):
                    hsl = slice(j * HS, (j + 1) * HS)
                    mm = nc.tensor.matmul(
                        stt[:, j * 512 : (j + 1) * 512],
                        k2[hp][hsl, ssl],
                        q2[hp][hsl, tsl],
                        start=True,
                        stop=True,
                    )
                    for key in (("q", hp, tq), ("k", hp, si // 4)):
                        cp = qk_cp.get(key)
                        if cp is not None:
                            tile.add_dep_helper(mm.ins, cp.ins, reason="qk chunk")
                return stt

            # Filler schedule: sched[b][si] = list of thunks issued on the PE
            # (or DVE/GpSimd for norms) inside iteration si of block b,
            # between the hoisted ST(si+1) and the AV(si) matmuls. Every item
            # is placed ahead of its consumer's deadline:
            #   k chunk c of the running pair  -> before ST(4c) issues (iter 4c-1)
            #   v tile si                      -> before AV(si) (iter si)
            #   q/k chunk 0 of the next pair   -> before the next block's ST(0)
            #   q chunk tq                     -> before block (tq,hp) starts
            #   norm of block b-1              -> iter 0 (frees its PSUM pair)
            #   proj tiles of t-chunk tq       -> after both norms of tq
            sched = [dict() for _ in range(len(blocks))]

            def put(b, it, fn, *args):
                sched[b].setdefault(it, []).append((fn, args))

            # block 0: v tiles + remaining k chunks of pair 0, then pair 1
            put(0, 0, emit_v_group, 0)
            put(0, 0, emit_v_group, 1)
            put(0, 0, emit_qk_group, "k", 0, 1)
            put(0, 1, emit_v_group, 2)
            put(0, 1, emit_v_group, 3)
            put(0, 2, emit_v_group, 4)
            put(0, 3, emit_qk_group, "k", 0, 2)
            put(0, 3, emit_v_group, 5)
            put(0, 4, emit_v_group, 6)
            put(0, 4, emit_v_group, 7)
            put(0, 5, emit_v_group, 8)
            put(0, 6, emit_v_group, 9)
            put(0, 7, emit_qk_group, "k", 0, 3)
            put(0, 7, emit_v_group, 10)
            put(0, 8, emit_v_group, 11)
            put(0, 9, emit_v_group, 12)
            put(0, 10, emit_v_group, 13)
            put(0, 11, emit_qk_group, "q", 1, 0)
            put(0, 12, emit_v_group, 14)
            put(0, 13, emit_qk_group, "k", 1, 0)
            put(0, 14, emit_v_group, 15)
            # block 1: rest of pair-1 k, q chunk for block 2
            put(1, 0, emit_norm, 0)
            put(1, 2, emit_qk_group, "k", 1, 1)
            put(1, 5, emit_qk_group, "k", 1, 2)
            put(1, 9, emit_qk_group, "k", 1, 3)
            put(1, 12, emit_qk_group, "q", 0, 1)
            # blocks 2..7: norms, projections (3+1 split keeps PE slack
            # even), and the remaining q chunks one block ahead of use
            # proj goes at iter >= 5: its explicit dep on the norm multiply
            # (which completes ~3 iterations after the block starts) would
            # otherwise head-of-line-block the in-order PE queue and stall
            # the exp stream at every block boundary.
            put(2, 0, emit_norm, 1)
            put(2, 5, emit_proj, 0)
            put(2, 8, emit_proj, 1)
            put(2, 11, emit_proj, 2)
            put(2, 13, emit_qk_group, "q", 1, 1)
            put(3, 0, emit_norm, 2)
            put(3, 5, emit_proj, 3)
            put(3, 8, emit_qk_group, "q", 0, 2)
            put(4, 0, emit_norm, 3)
            put(4, 5, emit_proj, 4)
            put(4, 8, emit_proj, 5)
            put(4, 11, emit_proj, 6)
            put(4, 13, emit_qk_group, "q", 1, 2)
            put(5, 0, emit_norm, 4)
            put(5, 5, emit_proj, 7)
            put(5, 8, emit_qk_group, "q", 0, 3)
            put(6, 0, emit_norm, 5)
            put(6, 5, emit_proj, 8)
            put(6, 8, emit_proj, 9)
            put(6, 11, emit_proj, 10)
            put(6, 13, emit_qk_group, "q", 1, 3)
            put(7, 0, emit_norm, 6)
            put(7, 5, emit_proj, 11)

            # Minimal prologue: only the chunk-0 q/k of pair 0, then straight
            # into the first score tile so the exp stream starts ~17us in.
            emit_qk_group("q", 0, 0)
            emit_qk_group("k", 0, 0)

            # software pipeline: issue ST(si+1) on the PE BEFORE the AV(si)
            # matmuls. AV(si) stalls the in-order PE queue on exp(si); with
            # ST(si+1) ahead of it, the array computes the next scores under
            # the ScalarE exp. The next BLOCK's ST(0) is likewise hoisted
            # into the current block's last iteration.
            stt = emit_st(blocks[0][1], blocks[0][0], 0)
            for bi, (tq, hp) in enumerate(blocks):
                tsl = slice(tq * 512, (tq + 1) * 512)
                av = [
                    ps_av.tile([2 * HS, 512], F32, tag="av", name=f"av{hp}_{tq}_{j}")
                    for j in range(2)
                ]
                avs[bi] = av
                for si in range(16):
                    att = attp.tile([P, 1024], F16, tag="att")
                    nc.scalar.activation(att, stt, func=EXP, scale=float(HS**-0.5))
                    if si < 15:
                        stt = emit_st(hp, tq, si + 1)
                    elif bi + 1 < len(blocks):
                        ntq, nhp = blocks[bi + 1]
                        stt = emit_st(nhp, ntq, 0)
                    for fn, args in sched[bi].get(si, ()):
                        fn(*args)
                    v3 = v_sb[si].rearrange("p (h e) -> p h e", h=HPC)
                    for j in range(2):
                        mm_av = nc.tensor.matmul(
                            av[j],
                            v3[:, 2 * hp + j, :],
                            att[:, j * 512 : (j + 1) * 512],
                            start=(si == 0),
                            stop=(si == 15),
                        )
                        for cp in v_cp.get(si, ()):
                            tile.add_dep_helper(mm_av.ins, cp.ins, reason="v tile")
                        if si == 15:
                            av_last[(bi, j)] = mm_av
            # drain: last block's norm + the final t-chunk's projection.
            # A few dummy matmuls keep the PE out of its low p-state while
            # the cross-engine norm chain runs, so the projection matmuls
            # execute at full clock. The dummies target a ps_st slot (free
            # once the last exp has read it) -- an "av" slot would stall on
            # the final accumulators.
            warm2 = ps_st.tile([P, 1024], F32, tag="st", name="warm2")
            for _ in range(5):
                nc.tensor.matmul(
                    warm2[:, 0:512], warm_in[:, 0:P], warm_in, start=True, stop=True
                )
            emit_norm(7, tail=True)
            for tt in range(12, 16):
                emit_proj(tt)


_NC_CACHE = None


def _combined_act_set_id() -> int:
    """Index (into act_info.json act_func_sets) of a set with exp."""
    try:
        import glob as _glob
        import json as _json
        import neuronxcc

        pat = os.path.join(
            os.path.dirname(neuronxcc.__file__), "pwp", "*", "act_info.json"
        )
        for p in sorted(_glob.glob(pat)):
            sets = _json.load(open(p))["act_func_sets"]
            for i, s in enumerate(sets):
                fns = s.get("act", {})
                if "exp" in fns and "ln" in fns:
                    return i
    except Exception:
        pass
    return 6  # natural_log_exp_and_others in the TRN2 act_info.json


def _dedupe_act_table_loads(nc):
    """Keep one ACT table load; drop the rest.

    Bacc's insert_act_table_loads can thrash (~2.7us per reload). Every
    activation we emit (Exp) lives in the combined set, so a single load
    up front is sufficient.
    """
    set_id = _combined_act_set_id()
    first = True
    for b in nc.m.functions[0].blocks:
        keep = []
        for inst in b.instructions:
            if isinstance(inst, mybir.InstLoadActFuncSet):
                if first:
                    inst.act_func_set_id = set_id
                    first = False
                    keep.append(inst)
            else:
                keep.append(inst)
        b.instructions[:] = keep


def _build():
    global _NC_CACHE
    if _NC_CACHE is not None:
        return _NC_CACHE
    nc = bacc.Bacc("TRN2", target_bir_lowering=False, debug=False, num_devices=NCORES)
    xT = nc.dram_tensor("xT", [D, T], F16, kind="ExternalInput").ap()
    wq = nc.dram_tensor("wq", [D, HPC * HS], F16, kind="ExternalInput").ap()
    wk = nc.dram_tensor("wk", [D, HPC * HS], F16, kind="ExternalInput").ap()
    wv = nc.dram_tensor("wv", [D, HPC * HS], F16, kind="ExternalInput").ap()
    wo = nc.dram_tensor("wo", [HPC * HS, D], F16, kind="ExternalInput").ap()
    bo2 = nc.dram_tensor("bo2", [1, D], F32, kind="ExternalInput").ap()
    y = nc.dram_tensor("y", [T, D], F16, kind="ExternalOutput").ap()
    with tile.TileContext(nc) as tc:
        _emit(tc, xT, wq, wk, wv, wo, bo2, y)
    nc.compile()
    _dedupe_act_table_loads(nc)
    _NC_CACHE = nc
    return nc


def _prep_in_maps(x, Wq, Wk, Wv, Wo, bo):
    x = np.asarray(x, dtype=np.float32)
    Wq = np.asarray(Wq, dtype=np.float32)
    Wk = np.asarray(Wk, dtype=np.float32)
    Wv = np.asarray(Wv, dtype=np.float32)
    Wo = np.asarray(Wo, dtype=np.float32)
    bo = np.asarray(bo, dtype=np.float32)
    in_maps = []
    for c in range(NCORES):
        b, hh = divmod(c, 2)
        hb = hh * HPC
        in_maps.append(
            {
                "xT": to_fp16(x[b].T),
                "wq": to_fp16(Wq[hb : hb + HPC].transpose(1, 0, 2).reshape(D, HPC * HS)),
                "wk": to_fp16(Wk[hb : hb + HPC].transpose(1, 0, 2).reshape(D, HPC * HS)),
                "wv": to_fp16(Wv[hb : hb + HPC].transpose(1, 0, 2).reshape(D, HPC * HS)),
                "wo": to_fp16(Wo[hb * HS : (hb + HPC) * HS, :]),
                "bo2": np.ascontiguousarray((0.5 * bo).reshape(1, D).astype(np.float32)),
            }
        )
    return in_maps


def _run(in_maps, trace=False):
    nc = _build()
    return run_bass_kernel_spmd(nc, in_maps, list(range(NCORES)), trace=trace)


def _run_prof(in_maps, tmpdir):
    nc = _build()
    return run_bass_kernel_spmd(
        nc, in_maps, list(range(NCORES)), trace=True, tmpdir=tmpdir
    )


def kernel(x, Wq, Wk, Wv, Wo, bo):
    in_maps = _prep_in_maps(x, Wq, Wk, Wv, Wo, bo)
    res = _run(in_maps)
    y = np.empty((B, T, D), dtype=np.float32)
    for b in range(B):
        y[b] = res.results[2 * b]["y"].astype(np.float32) + res.results[
            2 * b + 1
        ]["y"].astype(np.float32)
    return y
